# revision 9
# baseline (speedup 1.0000x reference)
"""Distributed Trainium2 kernel for AlternateWeaveGather (segment_reduce).

Reference computation:
    h = x @ W.T + b                      # [N, 512] linear
    out = segment_mean(h, batch, 256)    # [256, 512]

The linear layer commutes with the segment sum:
    out[s] = (segsum_x[s] @ W.T) / max(c[s], 1) + b * (c[s] > 0)

batch is sorted, so the host shards x at SEGMENT boundaries: rank j gets
exactly the rows of segments [32j, 32j+32), padded with zero rows to a
fixed P. Every rank then computes its 32 output rows entirely locally —
no collective, no cross-rank coupling. Segment counts are exact host-side
bincounts, shipped as 1/max(c,1) and b*(c>0).

The host ships x as bf16 (the device PE consumed x as truncated bf16
anyway; host-side round-to-nearest is strictly more accurate), halving
the HBM stream to ~16.6MB/core, and lays rows out so every aligned
4-row group belongs to a single segment (each segment zero-padded to a
multiple of 4; sums are row-order invariant). On-core per 512-row
superplane: DVE adds the 4 same-segment rows of each partition into one
row, then a single one-hot matmul accumulates segment sums into PSUM —
4x fewer PE passes than row-at-a-time. A tiny transpose + 512x512
linear epilogue finishes on-chip.
"""

import numpy as np

import concourse.bacc as bacc
import concourse.bass as bass
import concourse.mybir as mybir
import concourse.tile as tile
from concourse.bass_utils import run_bass_kernel_spmd

N_CORES = 8
N_ROWS = 131072
D = 512
N_SEG = 256
SEG_PER_CORE = N_SEG // N_CORES
P_MAIN = 16384          # 8 supertiles x 2048 rows
P_TAIL = 512            # one 512-row tail supertile
P = P_MAIN + P_TAIL     # padded rows per core
W_WIN = 64              # one-hot window (rel ids 0..31, trash=32)
TRASH = 32

F32 = mybir.dt.float32
I32 = mybir.dt.int32
BF16 = mybir.dt.bfloat16

N_SUP = P_MAIN // 2048  # 8 big supertiles (k=16 = 4 quads)
N_SPL = 8 * N_SUP + 2   # pair-planes (256 rows each)


def build_nc():
    nc = bacc.Bacc("TRN2", target_bir_lowering=False, debug=False,
                   num_devices=N_CORES)
    x = nc.dram_tensor("x", [P_MAIN, D], BF16, kind="ExternalInput")
    xt_d = nc.dram_tensor("xt_d", [P_TAIL, D], BF16, kind="ExternalInput")
    batchp = nc.dram_tensor("batchp", [128, N_SPL], F32,
                            kind="ExternalInput")
    wt = nc.dram_tensor("wt", [128, 4 * D], BF16, kind="ExternalInput")
    inv_d = nc.dram_tensor("inv_d", [SEG_PER_CORE, 1], F32,
                           kind="ExternalInput")
    bind_d = nc.dram_tensor("bind_d", [SEG_PER_CORE, D], F32,
                            kind="ExternalInput")
    out = nc.dram_tensor("out", [SEG_PER_CORE, D], F32, kind="ExternalOutput")

    iota_c = nc.inline_tensor(
        np.tile(np.arange(W_WIN, dtype=np.float32), (128, 1)).astype(
            mybir.dt.np(BF16)), name="iota_c")
    sel_c = nc.inline_tensor(
        np.eye(W_WIN, SEG_PER_CORE, dtype=np.float32).astype(
            mybir.dt.np(BF16)), name="sel_c")

    # [t, p, k, d]; per (t, p) the (16, 512) block is 16KB contiguous
    x_r = x.ap().rearrange("(t p k) d -> t p k d", p=128, k=16)
    xt_r = xt_d.ap().rearrange("(p k) d -> p k d", k=4)

    with tile.TileContext(nc) as tc:
        with tc.tile_pool(name="const", bufs=1) as const:
            iota_sb = const.tile([128, W_WIN], BF16, name="iota_sb")
            batch_sb = const.tile([128, N_SPL], F32, name="batch_sb")
            wt_sb = const.tile([128, 4 * D], BF16, name="wt_sb")
            sel_sb = const.tile([W_WIN, SEG_PER_CORE], BF16, name="sel_sb")
            inv_sb = const.tile([SEG_PER_CORE, 1], F32, name="inv_sb")
            bind_sb = const.tile([SEG_PER_CORE, D], F32, name="bind_sb")
            # stream-critical consts drain instantly at the sync-queue
            # head (tiny descriptors BEFORE the 16KB stream descriptors
            # queue up — behind them they trickle one per round-robin
            # turn). Epilogue-only consts load at the end.
            nc.sync.dma_start(out=iota_sb[:, :], in_=iota_c[:, :])
            nc.sync.dma_start(out=batch_sb[:, :], in_=batchp[:, :])

            with tc.tile_pool(name="xin", bufs=4) as xp, \
                 tc.tile_pool(name="xsum", bufs=3) as xsp, \
                 tc.tile_pool(name="ohp", bufs=4) as ohp, \
                 tc.tile_pool(name="psum_acc", bufs=1, space="PSUM") as pacc:
                ps = pacc.tile([W_WIN, D], F32, name="ps")
                qs = [nc.sync, nc.scalar]
                nq = 0

                def is_eq_mm(xs_q, col, start, stop):
                    oh = ohp.tile([128, W_WIN], BF16, name="oh")
                    nc.vector.tensor_scalar(
                        oh[:, :], iota_sb[:, :],
                        batch_sb[:, col:col + 1],
                        None, mybir.AluOpType.is_equal)
                    nc.tensor.matmul(ps[:, :], oh[:, :], xs_q,
                                     start=start, stop=stop,
                                     skip_group_check=True)

                def pair_add(xtile, xs, off):
                    # xs[:, m, :] = xtile[:, m, :] + xtile[:, m+off, :]
                    # — contiguous halves, full 2-elem/cycle bf16 rate
                    nc.vector.tensor_tensor(
                        xs[:, :, :], xtile[:, 0:off, :],
                        xtile[:, off:2 * off, :], mybir.AluOpType.add)

                for t in range(N_SUP):
                    xt = xp.tile([128, 16, D], BF16, name="xt")
                    if t == N_SUP - 1:
                        # split the final big supertile so the pipeline
                        # drains per-4-plane, not per-16-plane
                        for c in range(4):
                            nc.scalar.dma_start(
                                out=xt[:, 4 * c:4 * c + 4, :],
                                in_=x_r[t][:, 4 * c:4 * c + 4, :])
                            xs = xsp.tile([128, 2, D], BF16, name="xs",
                                          tag="xs")
                            pair_add(xt[:, 4 * c:4 * c + 4, :], xs, 2)
                            for m in range(2):
                                is_eq_mm(xs[:, m, :], 8 * t + 2 * c + m,
                                         False, False)
                    else:
                        qs[t % 2].dma_start(out=xt[:, :, :], in_=x_r[t])
                        xs = xsp.tile([128, 8, D], BF16, name="xs", tag="xs")
                        pair_add(xt[:, :, :], xs, 8)
                        for m in range(8):
                            is_eq_mm(xs[:, m, :], 8 * t + m,
                                     t == 0 and m == 0, False)

                # 512-row tail supertile (padded rows have rel id TRASH)
                xtl = xp.tile([128, 4, D], BF16, name="xtl")
                nc.sync.dma_start(out=xtl[:, :, :], in_=xt_r[:, :, :])
                xs = xsp.tile([128, 2, D], BF16, name="xs", tag="xs")
                pair_add(xtl[:, :, :], xs, 2)
                for m in range(2):
                    is_eq_mm(xs[:, m, :], 8 * N_SUP + m, False, m == 1)

                # epilogue-only consts (overlap the pipeline drain)
                nc.scalar.dma_start(out=sel_sb[:, :], in_=sel_c[:, :])
                nc.scalar.dma_start(out=inv_sb[:, :], in_=inv_d[:, :])
                nc.scalar.dma_start(out=bind_sb[:, :], in_=bind_d[:, :])
                nc.scalar.dma_start(out=wt_sb[:, :], in_=wt[:, :])

                with tc.tile_pool(name="epi", bufs=1) as epi, \
                     tc.tile_pool(name="psum_epi", bufs=1,
                                  space="PSUM") as pepi:
                    # segment sums live in ps rows 0..31 (32=trash,
                    # 33..63 exact zeros); truncate to bf16 in SBUF
                    sb_bf = epi.tile([W_WIN, D], BF16, name="sb_bf")
                    nc.vector.tensor_copy(sb_bf[:, :], ps[:, :])

                    # transpose via sel matmul: pt_c[d_c, s] =
                    #   sum_p sb_bf[p, d_c] * (p == s)
                    lhsT = epi.tile([128, 4 * SEG_PER_CORE], BF16,
                                    name="lhsT")
                    for c in range(4):
                        pt = pepi.tile([128, SEG_PER_CORE], F32, name="pt",
                                       tag="pt", bufs=2)
                        nc.tensor.matmul(pt[:, :],
                                         sb_bf[:, c * 128:(c + 1) * 128],
                                         sel_sb[:, :], start=True, stop=True)
                        eng_copy = (nc.vector.tensor_copy if c % 2 == 0
                                    else nc.scalar.copy)
                        eng_copy(
                            lhsT[:, c * SEG_PER_CORE:(c + 1) * SEG_PER_CORE],
                            pt[:, :])

                    po = pepi.tile([SEG_PER_CORE, D], F32, name="po")
                    for c in range(4):
                        nc.tensor.matmul(
                            po[:, :],
                            lhsT[:, c * SEG_PER_CORE:(c + 1) * SEG_PER_CORE],
                            wt_sb[:, c * D:(c + 1) * D],
                            start=(c == 0), stop=(c == 3))
                    res = epi.tile([SEG_PER_CORE, D], F32, name="res")
                    # res = (sums @ Wt) * inv + b*(c>0); split in column
                    # halves so the first output DMA overlaps the second
                    # half's compute
                    h = D // 2
                    nc.vector.scalar_tensor_tensor(
                        res[:, 0:h], po[:, 0:h], inv_sb[:, 0:1],
                        bind_sb[:, 0:h], mybir.AluOpType.mult,
                        mybir.AluOpType.add)
                    nc.sync.dma_start(out=out[:, 0:h], in_=res[:, 0:h])
                    nc.vector.scalar_tensor_tensor(
                        res[:, h:D], po[:, h:D], inv_sb[:, 0:1],
                        bind_sb[:, h:D], mybir.AluOpType.mult,
                        mybir.AluOpType.add)
                    nc.scalar.dma_start(out=out[:, h:D], in_=res[:, h:D])
    nc.compile()
    return nc


def make_in_maps(x, W, b, batch):
    x = np.asarray(x, dtype=np.float32)
    W = np.asarray(W, dtype=np.float32)
    b = np.asarray(b, dtype=np.float32)
    batch = np.asarray(batch).astype(np.int64)
    npbf = mybir.dt.np(BF16)
    xbf = x.astype(npbf)
    wtT = W.T.astype(npbf)
    wt = np.ascontiguousarray(np.concatenate(
        [wtT[i * 128:(i + 1) * 128] for i in range(4)], axis=1))
    counts = np.bincount(batch, minlength=N_SEG).astype(np.float32)
    bounds = np.searchsorted(batch, np.arange(N_SEG + 1))

    in_maps = []
    for j in range(N_CORES):
        # rows of segments [32j, 32j+32), each segment zero-padded to a
        # multiple of 4 so every aligned 4-row group is single-segment
        xj = np.zeros((P, D), dtype=npbf)
        rel = np.full((P,), TRASH, dtype=np.float32)
        pos = 0
        for s in range(j * SEG_PER_CORE, (j + 1) * SEG_PER_CORE):
            lo, hi = int(bounds[s]), int(bounds[s + 1])
            n = hi - lo
            np4 = -(-n // 4) * 4
            assert pos + np4 <= P, f"core {j}: padded rows exceed {P}"
            xj[pos:pos + n] = xbf[lo:hi]
            rel[pos:pos + np4] = s - j * SEG_PER_CORE
            pos += np4
        # distribute pair members into the two contiguous halves of
        # each DMA unit: full supertiles t=0..6 ([128,16,D] halves of 8
        # planes), t=7 as 4 sub-units of 4 planes (halves of 2), tail
        # unit of 4 planes (halves of 2). Pair i = padded rows (2i,2i+1).
        xa, xb = xj[0::2], xj[1::2]            # pair members [P//2, D]
        prel = rel[0::2]                       # pair rel ids
        xout = np.empty((P, D), dtype=npbf)
        cols = []
        # full supertiles t=0..6: pair (p, m) -> planes (m, m+8)
        for t in range(N_SUP - 1):
            base = t * 1024
            ua = xa[base:base + 1024].reshape(128, 8, D)
            ub = xb[base:base + 1024].reshape(128, 8, D)
            xout[t * 2048:(t + 1) * 2048] = np.concatenate(
                [ua, ub], axis=1).reshape(2048, D)
            rr = prel[base:base + 1024].reshape(128, 8)
            cols += [rr[:, m] for m in range(8)]
        # supertile 7 is DMA'd as 4 sub-units of planes 4c..4c+3, but
        # the device still indexes rows as 14336 + 16p + kk: sub c's
        # pair (p, m) -> planes (4c+m, 4c+2+m)
        x7 = np.empty((128, 16, D), dtype=npbf)
        for c in range(4):
            base = 7 * 1024 + c * 256
            x7[:, 4 * c:4 * c + 2] = xa[base:base + 256].reshape(128, 2, D)
            x7[:, 4 * c + 2:4 * c + 4] = \
                xb[base:base + 256].reshape(128, 2, D)
            rr = prel[base:base + 256].reshape(128, 2)
            cols += [rr[:, m] for m in range(2)]
        xout[7 * 2048:P_MAIN] = x7.reshape(2048, D)
        # tail unit: row = 16384 + 4p + kk; pair (p, m) -> (m, 2+m)
        base = P_MAIN // 2
        ua = xa[base:base + 256].reshape(128, 2, D)
        ub = xb[base:base + 256].reshape(128, 2, D)
        xout[P_MAIN:] = np.concatenate([ua, ub], axis=1).reshape(512, D)
        rr = prel[base:base + 256].reshape(128, 2)
        cols += [rr[:, m] for m in range(2)]
        xj = xout
        bp = np.stack(cols, axis=1)

        cj = counts[j * SEG_PER_CORE:(j + 1) * SEG_PER_CORE]
        inv = (1.0 / np.maximum(cj, 1.0)).reshape(SEG_PER_CORE, 1)
        bind = (cj > 0).astype(np.float32)[:, None] * b[None, :]
        in_maps.append({
            "x": np.ascontiguousarray(xj[:P_MAIN]),
            "xt_d": np.ascontiguousarray(xj[P_MAIN:]),
            "batchp": np.ascontiguousarray(bp.astype(np.float32)),
            "wt": wt,
            "inv_d": np.ascontiguousarray(inv.astype(np.float32)),
            "bind_d": np.ascontiguousarray(bind.astype(np.float32)),
        })
    return in_maps


_NC_CACHE = {}


def kernel(x, W, b, batch, num_segments, trace=False, trace_cores=None):
    assert int(num_segments) == N_SEG
    if "nc" not in _NC_CACHE:
        _NC_CACHE["nc"] = build_nc()
    nc = _NC_CACHE["nc"]
    in_maps = make_in_maps(x, W, b, batch)
    kw = {}
    if trace_cores is not None:
        kw["trace_cores"] = trace_cores
    res = run_bass_kernel_spmd(nc, in_maps, core_ids=list(range(N_CORES)),
                               trace=trace, **kw)
    full = np.concatenate([res.results[j]["out"] for j in range(N_CORES)],
                          axis=0)
    if trace:
        return full, res
    return full
